# revision 1
# baseline (speedup 1.0000x reference)
"""Trainium2 Bass kernel for nn_Attn: out = softmax(v . (W @ q_s + b)) over s.

Key algebraic identity:
    energies[s] = v . (W @ q[s] + b) = q[s] . (W^T v) + (v . b)
The (v . b) term is constant across s and softmax is shift-invariant, so it
drops out. The kernel therefore computes u = W^T v (tiny), then a matvec
energies = question @ u, then a global softmax. This is memory-bound:
question (128 MiB fp32) must stream from HBM once; everything else is noise.

Distribution over 8 NeuronCores — hidden-dim (h) sharding:
  - question is transposed on the host once (QT = question.T, [H, S]); core r
    gets the contiguous row-slab QT[r*128:(r+1)*128]  (a free view)
  - core r computes its u-slice u_r = W[:, r*128:(r+1)*128]^T v locally
    (W column-sharded, v replicated) — no cross-core traffic
  - core r computes PARTIAL energies e_r[s] = sum_{h in slab} u_r[h] QT[h,s]
    for ALL 32768 tokens on TensorE. fp32 matmuls are 2-pass (LOW/HIGH), so
    four M=1 matmuls are packed into the four 32-column groups of the PE
    array (tile_position via psum base partitions 0/32/64/96) to overlap.
  - ReduceScatter(add) + AllGather (cheaper than one AllReduce) give every
    core the summed energy vector; each computes the identical global softmax
    with per-partition max/sum stats (no cross-partition broadcasts needed
    until a single tiny transpose at the end); host takes core 0's output.
"""

import numpy as np

S = 32768
H = 1024
NCORES = 8
HC = H // 128  # 8 chunks of 128 for the u computation
NG = S // 512  # 64 energy groups of 512 tokens
GPT = 16  # groups per question tile
NT = NG // GPT  # 4 question tiles of [128, 8192] (4 MB DMAs)

_cached = {}


def _build():
    """Build + compile the SPMD Bass module (same NEFF on all 8 cores)."""
    from contextlib import ExitStack

    import concourse.bass as bass
    import concourse.mybir as mybir
    import concourse.tile as tile
    from concourse import bacc
    from concourse.masks import make_identity

    f32 = mybir.dt.float32
    AX = mybir.AxisListType
    OP = mybir.AluOpType
    ds = bass.ds

    nc = bacc.Bacc(
        "TRN2", target_bir_lowering=False, debug=False, num_devices=NCORES
    )

    qt = nc.dram_tensor("qt", [128, S], f32, kind="ExternalInput")
    wc = nc.dram_tensor("wc", [H, 128], f32, kind="ExternalInput")
    vt = nc.dram_tensor("vt", [128, HC], f32, kind="ExternalInput")
    out = nc.dram_tensor("out", [S], f32, kind="ExternalOutput")

    rg = [list(range(NCORES))]

    with tile.TileContext(nc) as tc, ExitStack() as ctx:
        const = ctx.enter_context(tc.tile_pool(name="const", bufs=1))
        qpool = ctx.enter_context(tc.tile_pool(name="qpool", bufs=NT))
        work = ctx.enter_context(tc.tile_pool(name="work", bufs=1))
        psum_e = ctx.enter_context(tc.tile_pool(name="psum_e", bufs=3, space="PSUM"))
        psum_s = ctx.enter_context(tc.tile_pool(name="psum_s", bufs=2, space="PSUM"))
        dram = ctx.enter_context(tc.tile_pool(name="dram", bufs=1, space="DRAM"))

        # ---- local u-slice: u_r[j] = sum_o W[o, r*128+j] v[o], j in [0,128) ----
        v_sb = const.tile([128, HC], f32)
        nc.sync.dma_start(v_sb[:], vt[:])
        wc_sb = const.tile([128, HC, 128], f32)
        nc.sync.dma_start(wc_sb[:], wc[:].rearrange("(c p) m -> p c m", p=128))
        pu = psum_s.tile([128, 1], f32, tag="stat")
        for c in range(HC):
            nc.tensor.matmul(
                pu[:], lhsT=wc_sb[:, c, :], rhs=v_sb[:, c : c + 1],
                start=(c == 0), stop=(c == HC - 1),
            )
        u_loc = const.tile([128, 1], f32)
        nc.vector.tensor_copy(u_loc[:], pu[:])

        # ---- partial energies for ALL tokens over this core's h-slab ----
        # 4 token-groups of 512 per PSUM bank, one per PE column-group: the
        # four M=1 fp32 matmuls overlap in the array (tile_position derives
        # from the psum slice's base partition 0/32/64/96).
        e_loc_dram = dram.tile([1, S], f32)
        e_view = e_loc_dram[:].rearrange("one (g s) -> (one g) s", s=512)
        for t in range(NT):
            q_sb = qpool.tile([128, GPT * 512], f32, tag="q")
            nc.sync.dma_start(q_sb[:], qt[:, ds(t * GPT * 512, GPT * 512)])
            for quad in range(GPT // 4):
                pe4 = psum_e.tile([128, 512], f32, tag="pe4")
                pe4_rows = pe4[:].rearrange("(a b) s -> a b s", b=32)
                for j in range(4):
                    nc.tensor.matmul(
                        pe4[32 * j : 32 * j + 1, :],
                        lhsT=u_loc[:],
                        rhs=q_sb[:, ds((quad * 4 + j) * 512, 512)],
                        start=True, stop=True,
                        tile_position=(0, 32 * j),
                    )
                g0 = t * GPT + quad * 4
                # engines can't read strided partitions; copy the full bank
                # (same cost — free-dim bound) and let the DMA stride instead
                e_sb = work.tile([128, 512], f32, tag="esb", bufs=4)
                if quad % 2 == 0:
                    nc.scalar.copy(e_sb[:], pe4[:])
                else:
                    nc.vector.tensor_copy(e_sb[:], pe4[:])
                e_sb_rows = e_sb[:].rearrange("(a b) s -> a b s", b=32)
                # SWDGE queue: keep exports off the Sync queue so they never
                # serialize against the big qt tile loads
                nc.gpsimd.dma_start(e_view[g0 : g0 + 4, :], e_sb_rows[:, 0, :])

        # ---- sum partials across cores: ReduceScatter + AllGather ----
        rs_out = dram.tile([1, S // NCORES], f32)
        nc.gpsimd.collective_compute(
            "ReduceScatter", OP.add, replica_groups=rg,
            ins=[e_loc_dram[:].rearrange("one (r s) -> (one r) s", r=NCORES).opt()],
            outs=[rs_out.opt()],
        )
        e_sum_dram = dram.tile([NCORES, S // NCORES], f32)
        nc.gpsimd.collective_compute(
            "AllGather", OP.bypass, replica_groups=rg,
            ins=[rs_out.opt()], outs=[e_sum_dram.opt()],
        )

        # ---- global softmax over all 32768 energies ----
        # Layout [128, 256]: per-partition stats first (no broadcasts), then
        # one tiny transpose to combine across partitions and one to come back.
        F = S // 128  # 256
        e_all = work.tile([128, F], f32)
        nc.sync.dma_start(
            e_all[:], e_sum_dram[:].rearrange("r (q f) -> (r q) f", f=F)
        )
        negrow = work.tile([128, 1], f32)
        nc.vector.tensor_reduce(negrow[:], e_all[:], axis=AX.X, op=OP.max, negate=True)
        ex1 = work.tile([128, F], f32)
        rowsum = work.tile([128, 1], f32)
        nc.scalar.activation(
            ex1[:], e_all[:], mybir.ActivationFunctionType.Exp,
            bias=negrow[:], scale=1.0, accum_out=rowsum[:],
        )
        ident = const.tile([128, 128], f32)
        make_identity(nc, ident[:])
        ptr_a = psum_s.tile([1, 128], f32, tag="stat")
        nc.tensor.transpose(ptr_a[:], negrow[:], ident[:])
        ptr_b = psum_s.tile([1, 128], f32, tag="statb")
        nc.tensor.transpose(ptr_b[:], rowsum[:], ident[:])
        tp0 = work.tile([1, 128], f32)
        nc.vector.tensor_copy(tp0[:], ptr_a[:])
        tp1 = work.tile([1, 128], f32)
        nc.scalar.copy(tp1[:], ptr_b[:])
        # global stats on one partition: m = max_j rowmax_j, s = sum_j
        # rowsum_j * exp(rowmax_j - m); tp0 holds -rowmax_j, tp1 rowsum_j
        negm = work.tile([1, 1], f32)
        nc.vector.tensor_reduce(negm[:], tp0[:], axis=AX.X, op=OP.min)
        texp = work.tile([1, 128], f32)
        nc.scalar.activation(
            texp[:], tp0[:], mybir.ActivationFunctionType.Exp,
            bias=negm[:], scale=-1.0,
        )
        prod = work.tile([1, 128], f32)
        nc.vector.tensor_mul(prod[:], texp[:], tp1[:])
        stot = work.tile([1, 1], f32)
        nc.vector.tensor_reduce(stot[:], prod[:], axis=AX.X, op=OP.add)
        rtot = work.tile([1, 1], f32)
        nc.vector.reciprocal(rtot[:], stot[:])
        # K=1 matmul does transpose + scale in one: scl[j] = texp[j] / s
        pscl = psum_s.tile([128, 1], f32, tag="stat")
        nc.tensor.matmul(pscl[:], lhsT=texp[:], rhs=rtot[:], start=True, stop=True)
        scl = work.tile([128, 1], f32)
        nc.vector.tensor_copy(scl[:], pscl[:])
        outt = work.tile([128, F], f32)
        nc.vector.tensor_scalar_mul(outt[:], ex1[:], scl[:])
        nc.sync.dma_start(out[:].rearrange("(p f) -> p f", f=F), outt[:])

    nc.compile()
    return nc


def _get_nc():
    if "nc" not in _cached:
        _cached["nc"] = _build()
    return _cached["nc"]


def make_in_maps(question, W, v):
    q = np.ascontiguousarray(np.asarray(question, dtype=np.float32))
    Wn = np.ascontiguousarray(np.asarray(W, dtype=np.float32))
    vn = np.ascontiguousarray(np.asarray(v, dtype=np.float32))
    qtf = np.ascontiguousarray(q.T)  # [H, S]; row-slabs below are free views
    vt = np.ascontiguousarray(vn.reshape(HC, 128).T)  # [128, HC]
    in_maps = []
    for r in range(NCORES):
        in_maps.append(
            {
                "qt": qtf[r * 128 : (r + 1) * 128],
                "wc": np.ascontiguousarray(Wn[:, r * 128 : (r + 1) * 128]),
                "vt": vt,
            }
        )
    return in_maps


def run(question, W, v, **spmd_kwargs):
    """Run the SPMD kernel; returns (out [S] fp32, BassKernelResults)."""
    from concourse.bass_utils import run_bass_kernel_spmd

    nc = _get_nc()
    in_maps = make_in_maps(question, W, v)
    res = run_bass_kernel_spmd(nc, in_maps, core_ids=list(range(NCORES)), **spmd_kwargs)
    return np.asarray(res.results[0]["out"], dtype=np.float32), res


def kernel(question, W, b, v):
    out, _ = run(question, W, v)
    return out.reshape(1, 1, S)

